# revision 34
# baseline (speedup 1.0000x reference)
"""Trainium2 Bass kernel for nn_AtomicLinear.

Math: reference computes (x[:, None, :] * weight)[:, :, indices].sum(2) + bias,
which equals sum_i x[b, idx[i]] * w[o, idx[i]] = sum_j c_j * x[b, j] * w[o, j]
with c_j = multiplicity of j in indices.  So the whole op is a plain GEMM
against a count-scaled weight:  out = x @ (weight * c).T + bias.

Device strategy: shard batch into BG groups x out_features into OG groups
(BG*OG = 8 cores).  Each core computes out_blk.T = W_t.T @ x_blk.T with
W_t = (weight * c).T [in, out/OG] (host pre-transposed), x_blk.T [in, B/BG]
(host pre-transposed).  All DMAs are large contiguous-chunk slab loads:
  lhsT = W_t tile  [K=128 in, M=128 out]   (stationary)
  rhs  = x.T tile  [K=128 in, N=512 batch] (moving)
  psum [M=128 out, N=512 batch], accumulated over 16 K-tiles.
Bias is folded in during the PSUM->SBUF eviction on the scalar engine.
"""

import numpy as np

B = 4096
IN_F = 2048
OUT_F = 2048
N_CORES = 8

KT = IN_F // 128  # 16 contraction tiles

LAST_RESULTS = None

PE_DTYPE = "float16"  # 1 cycle/row on PE; rel err ~2.9e-4
# 8x1 batch-only grid measured faster and more stable than 4x2 in paired
# A/B runs (medians ~5.7ms vs ~6.9ms per 52-repeat batch, both orderings)
BG = 8  # batch groups
OG = 1  # out-feature groups

_prog_cache = {}


def _shapes(bg, og):
    b_sh = B // bg
    out_sh = OUT_F // og
    mt = out_sh // 128
    nt = b_sh // 512
    return b_sh, out_sh, mt, nt


def _build_program(pe_dtype_name=None, bg=None, og=None, repeats=1, w_bufs=16,
                   xch=4, x_eng="scalar", ps_bufs=8, evict="act",
                   out_eng="scalar", w_split=1, layout="stream",
                   out_fp16=False, wch=4, loop_order="kphase"):
    import concourse.tile as tile
    from concourse import bacc, mybir

    pe_dtype_name = pe_dtype_name or PE_DTYPE
    bg = bg or BG
    og = og or OG

    key = (pe_dtype_name, bg, og, repeats, w_bufs, xch, x_eng, ps_bufs,
           evict, out_eng, w_split, layout, out_fp16, wch, loop_order)
    if key in _prog_cache:
        return _prog_cache[key]
    if layout == "packed":
        nc = _build_packed(pe_dtype_name, bg, og, repeats, xch, x_eng,
                           ps_bufs, out_eng, out_fp16, wch, loop_order)
        _prog_cache[key] = nc
        return nc

    b_sh, out_sh, mt, nt = _shapes(bg, og)
    pe_dt = getattr(mybir.dt, pe_dtype_name)
    f32 = mybir.dt.float32

    nc = bacc.Bacc(
        "TRN2", target_bir_lowering=False, debug=False, num_devices=N_CORES
    )

    xt_d = nc.dram_tensor("xt", [IN_F, b_sh], pe_dt, kind="ExternalInput").ap()
    wt_d = nc.dram_tensor("wt", [IN_F, out_sh], pe_dt, kind="ExternalInput").ap()
    bias_d = nc.dram_tensor("biaspm", [128, mt], f32, kind="ExternalInput").ap()
    out_d = nc.dram_tensor("outT", [out_sh, b_sh], f32, kind="ExternalOutput").ap()

    XCH = xch  # x loaded in XCH chunks so PE can start before the full load

    with tile.TileContext(nc) as tc:
        with (
            tc.tile_pool(name="xsb", bufs=2) as xpool,
            tc.tile_pool(name="wsb", bufs=w_bufs) as wpool,
            tc.tile_pool(name="bsb", bufs=1) as bpool,
            tc.tile_pool(name="osb", bufs=4) as opool,
            tc.tile_pool(name="ps", bufs=ps_bufs, space="PSUM") as pspool,
        ):
            bias_sb = bpool.tile([128, mt], f32)
            bias_loaded = [False]

            def _load_bias():
                # deferred so the bias DMA doesn't occupy the SDMA pool
                # ahead of the critical first x/w transfers (bias is first
                # needed at the first psum eviction, ~20us in)
                if not bias_loaded[0]:
                    bias_loaded[0] = True
                    nc.gpsimd.dma_start(out=bias_sb[:], in_=bias_d[:])

            # dram views with 128-partition tiling folded out
            xt_v = xt_d.rearrange("(t p) f -> p t f", p=128)  # [128, KT, b_sh]
            wt_v = wt_d.rearrange("(t p) f -> p t f", p=128)  # [128, KT, out_sh]

            kc = KT // XCH

            def _mm(ps, w_m, m, n, k):
                # x chunks are separate tiles so matmuls depend only on the
                # chunk that holds their k-tile (deps are bank-granular)
                ci, kl = divmod(k, kc)
                nc.tensor.matmul(
                    ps[:],
                    lhsT=w_m[:, k * 128 : (k + 1) * 128],
                    rhs=x_cs[ci][
                        :, kl * b_sh + n * 512 : kl * b_sh + (n + 1) * 512
                    ],
                    start=(k == 0),
                    stop=(k == KT - 1),
                )

            def _evict(ps, m, n):
                ot = opool.tile([128, 512], f32, name=f"ot{m}_{n}", tag="ot")
                use_dve = evict == "dve" or (
                    evict == "split" and (m * nt + n) % 2 == 1
                )
                if use_dve:
                    nc.vector.tensor_scalar_add(
                        ot[:], ps[:], bias_sb[:, m : m + 1]
                    )
                else:
                    nc.scalar.activation(
                        ot[:],
                        ps[:],
                        mybir.ActivationFunctionType.Identity,
                        bias=bias_sb[:, m : m + 1],
                    )
                getattr(nc, out_eng).dma_start(
                    out=out_d[
                        m * 128 : (m + 1) * 128, n * 512 : (n + 1) * 512
                    ],
                    in_=ot[:],
                )

            def _load_w(m):
                w_m = wpool.tile([128, KT * 128], pe_dt, name=f"wm{m}", tag="wm")
                w_v = w_m[:].rearrange("p (t f) -> p t f", t=KT)
                wkc = KT // w_split
                for i in range(w_split):
                    nc.sync.dma_start(
                        out=w_v[:, i * wkc : (i + 1) * wkc, :],
                        in_=wt_v[
                            :, i * wkc : (i + 1) * wkc,
                            m * 128 : (m + 1) * 128,
                        ],
                    )
                return w_m

            # repeats > 1 re-runs the identical computation inside one NEFF;
            # used for benchmarking (HW time = delta between repeat counts).
            for _rep in range(repeats):
                x_engine = getattr(nc, x_eng)
                x_cs = []
                for i in range(XCH):
                    x_c = xpool.tile(
                        [128, kc * b_sh], pe_dt, name=f"xc{i}", tag=f"xc{i}"
                    )
                    x_cs.append(x_c)
                    x_engine.dma_start(
                        out=x_c[:].rearrange("p (t f) -> p t f", t=kc),
                        in_=xt_v[:, i * kc : (i + 1) * kc, :],
                    )

                if loop_order == "kphase":
                    # Phase A: while x streams in, run the first PH m-tiles'
                    # partial-k accumulations chunk by chunk across PH psum
                    # banks, so the PE has PH*kc matmuls per arriving x chunk
                    # instead of kc.  Phase B: remaining m-tiles normally.
                    assert nt == 1
                    PH = min(mt, ps_bufs)
                    w_ts = {m: _load_w(m) for m in range(mt)}
                    _load_bias()
                    ps_l = [
                        pspool.tile([128, 512], f32, name=f"psA{m}", tag="ps")
                        for m in range(PH)
                    ]
                    for c in range(XCH):
                        for m in range(PH):
                            for k in range(c * kc, (c + 1) * kc):
                                _mm(ps_l[m], w_ts[m], m, 0, k)
                    for m in range(PH):
                        _evict(ps_l[m], m, 0)
                    for m in range(PH, mt):
                        ps = pspool.tile([128, 512], f32, name=f"psB{m}", tag="ps")
                        for k in range(KT):
                            _mm(ps, w_ts[m], m, 0, k)
                        _evict(ps, m, 0)
                else:
                    _load_bias()
                    for m in range(mt):
                        w_m = _load_w(m)
                        for n in range(nt):
                            ps = pspool.tile(
                                [128, 512], f32, name=f"psm{m}_{n}", tag="ps"
                            )
                            for k in range(KT):
                                _mm(ps, w_m, m, n, k)
                            _evict(ps, m, n)

    nc.compile()
    _prog_cache[key] = nc
    return nc


def _build_packed(pe_dtype_name, bg, og, repeats, xch, x_eng, ps_bufs,
                  out_eng, out_fp16, wch, loop_order="mnk"):
    """Variant with x and w host-packed into their exact SBUF layouts
    ([128, KT*free]) so every DMA moves contiguous >=2KB per-partition rows,
    and the whole per-core weight stays SBUF-resident (no per-m streaming)."""
    import concourse.tile as tile
    from concourse import bacc, mybir

    b_sh, out_sh, mt, nt = _shapes(bg, og)
    pe_dt = getattr(mybir.dt, pe_dtype_name)
    f32 = mybir.dt.float32
    out_dt = pe_dt if out_fp16 else f32

    nc = bacc.Bacc(
        "TRN2", target_bir_lowering=False, debug=False, num_devices=N_CORES
    )

    xt_d = nc.dram_tensor("xt", [128, KT * b_sh], pe_dt, kind="ExternalInput").ap()
    wt_d = nc.dram_tensor("wt", [128, KT * out_sh], pe_dt, kind="ExternalInput").ap()
    bias_d = nc.dram_tensor("biaspm", [128, mt], f32, kind="ExternalInput").ap()
    out_d = nc.dram_tensor("outT", [out_sh, b_sh], out_dt, kind="ExternalOutput").ap()

    with tile.TileContext(nc) as tc:
        with (
            tc.tile_pool(name="xsb", bufs=2) as xpool,
            tc.tile_pool(name="wsb", bufs=2) as wpool,
            tc.tile_pool(name="bsb", bufs=1) as bpool,
            tc.tile_pool(name="osb", bufs=4) as opool,
            tc.tile_pool(name="ps", bufs=ps_bufs, space="PSUM") as pspool,
        ):
            bias_sb = bpool.tile([128, mt], f32)
            nc.gpsimd.dma_start(out=bias_sb[:], in_=bias_d[:])

            for _rep in range(repeats):
                x_sb = xpool.tile([128, KT * b_sh], pe_dt)
                xc = KT * b_sh // xch
                x_engine = getattr(nc, x_eng)
                for i in range(xch):
                    x_engine.dma_start(
                        out=x_sb[:, i * xc : (i + 1) * xc],
                        in_=xt_d[:, i * xc : (i + 1) * xc],
                    )
                w_sb = wpool.tile([128, KT * out_sh], pe_dt)
                wc = KT * out_sh // wch
                for i in range(wch):
                    nc.sync.dma_start(
                        out=w_sb[:, i * wc : (i + 1) * wc],
                        in_=wt_d[:, i * wc : (i + 1) * wc],
                    )

                def _evict(ps, m, n):
                    ot = opool.tile([128, 512], out_dt)
                    nc.scalar.activation(
                        ot[:],
                        ps[:],
                        mybir.ActivationFunctionType.Identity,
                        bias=bias_sb[:, m : m + 1],
                    )
                    getattr(nc, out_eng).dma_start(
                        out=out_d[
                            m * 128 : (m + 1) * 128, n * 512 : (n + 1) * 512
                        ],
                        in_=ot[:],
                    )

                def _mm(ps, m, n, k):
                    nc.tensor.matmul(
                        ps[:],
                        lhsT=w_sb[
                            :, k * out_sh + m * 128 : k * out_sh + (m + 1) * 128
                        ],
                        rhs=x_sb[
                            :, k * b_sh + n * 512 : k * b_sh + (n + 1) * 512
                        ],
                        start=(k == 0),
                        stop=(k == KT - 1),
                    )

                for m in range(mt):
                    if loop_order == "mkn":
                        # same stationary weight feeds all n back-to-back
                        ps_list = [
                            pspool.tile([128, 512], f32, name=f"psn{n}", tag="ps")
                            for n in range(nt)
                        ]
                        for k in range(KT):
                            for n in range(nt):
                                _mm(ps_list[n], m, n, k)
                        for n in range(nt):
                            _evict(ps_list[n], m, n)
                    else:
                        for n in range(nt):
                            ps = pspool.tile([128, 512], f32)
                            for k in range(KT):
                                _mm(ps, m, n, k)
                            _evict(ps, m, n)

    nc.compile()
    return nc


def _pack_sbuf_layout(a, free):
    """[KT*128, free] -> [128, KT*free] matching the SBUF tile layout."""
    return np.ascontiguousarray(
        a.reshape(KT, 128, free).transpose(1, 0, 2).reshape(128, KT * free)
    )


def _prep_host(x, weight, bias, indices, pe_dtype_name=None, bg=None, og=None,
               layout="stream"):
    from concourse import mybir

    pe_dtype_name = pe_dtype_name or PE_DTYPE
    bg = bg or BG
    og = og or OG
    b_sh, out_sh, mt, nt = _shapes(bg, og)
    np_dt = mybir.dt.np(getattr(mybir.dt, pe_dtype_name))

    x = np.asarray(x, dtype=np.float32)
    weight = np.asarray(weight, dtype=np.float32)
    bias = np.asarray(bias, dtype=np.float32)
    idx = np.asarray(indices).astype(np.int64)

    counts = np.bincount(idx, minlength=IN_F).astype(np.float32)
    w_t = np.ascontiguousarray((weight * counts[None, :]).T).astype(np_dt)  # [in, out]
    xt_full = np.ascontiguousarray(x.T).astype(np_dt)  # [in, B]

    in_maps = []
    for c in range(N_CORES):
        bgi, ogi = divmod(c, og)
        bias_blk = bias[ogi * out_sh : (ogi + 1) * out_sh]
        xt_c = np.ascontiguousarray(xt_full[:, bgi * b_sh : (bgi + 1) * b_sh])
        wt_c = np.ascontiguousarray(w_t[:, ogi * out_sh : (ogi + 1) * out_sh])
        if layout == "packed":
            xt_c = _pack_sbuf_layout(xt_c, b_sh)
            wt_c = _pack_sbuf_layout(wt_c, out_sh)
        in_maps.append(
            {
                "xt": xt_c,
                "wt": wt_c,
                "biaspm": np.ascontiguousarray(bias_blk.reshape(mt, 128).T),
            }
        )
    return in_maps


def _gather_out(results, bg=None, og=None):
    bg = bg or BG
    og = og or OG
    b_sh, out_sh, mt, nt = _shapes(bg, og)
    out = np.empty((B, OUT_F), dtype=np.float32)
    for c in range(N_CORES):
        bgi, ogi = divmod(c, og)
        out[
            bgi * b_sh : (bgi + 1) * b_sh, ogi * out_sh : (ogi + 1) * out_sh
        ] = results[c]["outT"].T.astype(np.float32)
    return out


_runner_cache = {}


def _get_runner(nc):
    """Cached jitted executor for the compiled bass program (the
    run_bass_kernel_spmd axon path rebuilds its jit closure per call;
    caching avoids re-tracing on repeat kernel() invocations)."""
    if id(nc) in _runner_cache:
        return _runner_cache[id(nc)]

    import jax
    from jax.sharding import Mesh, PartitionSpec
    from jax.experimental.shard_map import shard_map
    import concourse.mybir as mybir
    from concourse.bass2jax import (
        _bass_exec_p,
        install_neuronx_cc_hook,
        partition_id_tensor,
    )

    install_neuronx_cc_hook()
    partition_name = nc.partition_id_tensor.name if nc.partition_id_tensor else None
    in_names, out_names, out_avals, zero_shapes = [], [], [], []
    for alloc in nc.m.functions[0].allocations:
        if not isinstance(alloc, mybir.MemoryLocationSet):
            continue
        name = alloc.memorylocations[0].name
        if alloc.kind == "ExternalInput":
            if name != partition_name:
                in_names.append(name)
        elif alloc.kind == "ExternalOutput":
            out_names.append(name)
            shape = tuple(alloc.tensor_shape)
            dtype = mybir.dt.np(alloc.dtype)
            out_avals.append(jax.core.ShapedArray(shape, dtype))
            zero_shapes.append((shape, dtype))
    all_in_names = list(in_names) + list(out_names)
    if partition_name is not None:
        all_in_names.append(partition_name)

    def _body(*args):
        operands = list(args)
        if partition_name is not None:
            operands.append(partition_id_tensor())
        return tuple(
            _bass_exec_p.bind(
                *operands,
                out_avals=tuple(out_avals),
                in_names=tuple(all_in_names),
                out_names=tuple(out_names),
                lowering_input_output_aliases=(),
                sim_require_finite=True,
                sim_require_nnan=True,
                nc=nc,
            )
        )

    devices = jax.devices()[:N_CORES]
    mesh = Mesh(np.asarray(devices), ("core",))
    n_io = len(in_names) + len(out_names)
    fn = jax.jit(
        shard_map(
            _body,
            mesh=mesh,
            in_specs=(PartitionSpec("core"),) * n_io,
            out_specs=(PartitionSpec("core"),) * len(out_names),
            check_rep=False,
        ),
        keep_unused=True,
    )

    runner = (fn, in_names, out_names, out_avals, zero_shapes)
    _runner_cache[id(nc)] = runner
    return runner


def _run_fast(nc, in_maps):
    fn, in_names, out_names, out_avals, zero_shapes = _get_runner(nc)
    concat_in = [
        np.concatenate([np.asarray(m[name]) for m in in_maps], axis=0)
        for name in in_names
    ]
    concat_zeros = [
        np.zeros((N_CORES * s[0], *s[1:]), dt) for s, dt in zero_shapes
    ]
    outs = fn(*concat_in, *concat_zeros)
    return [
        {
            name: np.asarray(outs[i]).reshape(N_CORES, *out_avals[i].shape)[c]
            for i, name in enumerate(out_names)
        }
        for c in range(N_CORES)
    ]


def kernel(x, weight, bias, indices):
    global LAST_RESULTS

    in_maps = _prep_host(x, weight, bias, indices)
    nc = _build_program()
    try:
        results = _run_fast(nc, in_maps)
    except Exception:
        from concourse.bass_utils import run_bass_kernel_spmd

        results = run_bass_kernel_spmd(
            nc, in_maps, list(range(N_CORES))
        ).results
    LAST_RESULTS = results
    return _gather_out(results)


# revision 42
# speedup vs baseline: 1.0214x; 1.0214x over previous
"""Trainium2 Bass kernel for nn_AtomicLinear.

Math: reference computes (x[:, None, :] * weight)[:, :, indices].sum(2) + bias,
which equals sum_i x[b, idx[i]] * w[o, idx[i]] = sum_j c_j * x[b, j] * w[o, j]
with c_j = multiplicity of j in indices.  So the whole op is a plain GEMM
against a count-scaled weight:  out = x @ (weight * c).T + bias.

Device strategy: shard batch into BG groups x out_features into OG groups
(BG*OG = 8 cores).  Each core computes out_blk.T = W_t.T @ x_blk.T with
W_t = (weight * c).T [in, out/OG] (host pre-transposed), x_blk.T [in, B/BG]
(host pre-transposed).  All DMAs are large contiguous-chunk slab loads:
  lhsT = W_t tile  [K=128 in, M=128 out]   (stationary)
  rhs  = x.T tile  [K=128 in, N=512 batch] (moving)
  psum [M=128 out, N=512 batch], accumulated over 16 K-tiles.
Bias is folded in during the PSUM->SBUF eviction on the scalar engine.
"""

import numpy as np

B = 4096
IN_F = 2048
OUT_F = 2048
N_CORES = 8

KT = IN_F // 128  # 16 contraction tiles

LAST_RESULTS = None

PE_DTYPE = "float16"  # 1 cycle/row on PE; rel err ~2.9e-4
# 8x1 batch-only grid measured faster and more stable than 4x2 in paired
# A/B runs (medians ~5.7ms vs ~6.9ms per 52-repeat batch, both orderings)
BG = 8  # batch groups
OG = 1  # out-feature groups

_prog_cache = {}


def _shapes(bg, og):
    b_sh = B // bg
    out_sh = OUT_F // og
    mt = out_sh // 128
    nt = b_sh // 512
    return b_sh, out_sh, mt, nt


def _build_program(pe_dtype_name=None, bg=None, og=None, repeats=1, w_bufs=16,
                   xch=4, x_eng="scalar", ps_bufs=8, evict="act",
                   out_eng="scalar", w_split=1, layout="pslab",
                   out_fp16=False, wch=4, loop_order="kphase"):
    import concourse.tile as tile
    from concourse import bacc, mybir

    pe_dtype_name = pe_dtype_name or PE_DTYPE
    bg = bg or BG
    og = og or OG

    key = (pe_dtype_name, bg, og, repeats, w_bufs, xch, x_eng, ps_bufs,
           evict, out_eng, w_split, layout, out_fp16, wch, loop_order)
    if key in _prog_cache:
        return _prog_cache[key]
    if layout == "packed":
        nc = _build_packed(pe_dtype_name, bg, og, repeats, xch, x_eng,
                           ps_bufs, out_eng, out_fp16, wch, loop_order)
        _prog_cache[key] = nc
        return nc

    b_sh, out_sh, mt, nt = _shapes(bg, og)
    pe_dt = getattr(mybir.dt, pe_dtype_name)
    f32 = mybir.dt.float32

    nc = bacc.Bacc(
        "TRN2", target_bir_lowering=False, debug=False, num_devices=N_CORES
    )

    if layout == "pslab":
        # host-packed: x in SBUF layout, w slab-major — every DMA moves
        # contiguous 4KB-per-partition runs (no 256B-chunk penalty)
        xt_d = nc.dram_tensor(
            "xt", [128, KT * b_sh], pe_dt, kind="ExternalInput"
        ).ap()
        wt_d = nc.dram_tensor(
            "wt", [mt * 128, KT * 128], pe_dt, kind="ExternalInput"
        ).ap()
    else:
        xt_d = nc.dram_tensor("xt", [IN_F, b_sh], pe_dt, kind="ExternalInput").ap()
        wt_d = nc.dram_tensor(
            "wt", [IN_F, out_sh], pe_dt, kind="ExternalInput"
        ).ap()
    bias_d = nc.dram_tensor("biaspm", [128, mt], f32, kind="ExternalInput").ap()
    out_d = nc.dram_tensor("outT", [out_sh, b_sh], f32, kind="ExternalOutput").ap()

    XCH = xch  # x loaded in XCH chunks so PE can start before the full load

    with tile.TileContext(nc) as tc:
        with (
            tc.tile_pool(name="xsb", bufs=2) as xpool,
            tc.tile_pool(name="wsb", bufs=w_bufs) as wpool,
            tc.tile_pool(name="bsb", bufs=1) as bpool,
            tc.tile_pool(name="osb", bufs=4) as opool,
            tc.tile_pool(name="ps", bufs=ps_bufs, space="PSUM") as pspool,
        ):
            bias_sb = bpool.tile([128, mt], f32)
            bias_loaded = [False]

            def _load_bias():
                # deferred so the bias DMA doesn't occupy the SDMA pool
                # ahead of the critical first x/w transfers (bias is first
                # needed at the first psum eviction, ~20us in)
                if not bias_loaded[0]:
                    bias_loaded[0] = True
                    nc.gpsimd.dma_start(out=bias_sb[:], in_=bias_d[:])

            # dram views with 128-partition tiling folded out
            if layout != "pslab":
                xt_v = xt_d.rearrange("(t p) f -> p t f", p=128)
                wt_v = wt_d.rearrange("(t p) f -> p t f", p=128)

            kc = KT // XCH

            def _mm(ps, w_m, m, n, k):
                # x chunks are separate tiles so matmuls depend only on the
                # chunk that holds their k-tile (deps are bank-granular)
                ci, kl = divmod(k, kc)
                nc.tensor.matmul(
                    ps[:],
                    lhsT=w_m[:, k * 128 : (k + 1) * 128],
                    rhs=x_cs[ci][
                        :, kl * b_sh + n * 512 : kl * b_sh + (n + 1) * 512
                    ],
                    start=(k == 0),
                    stop=(k == KT - 1),
                )

            def _evict(ps, m, n):
                ot = opool.tile([128, 512], f32, name=f"ot{m}_{n}", tag="ot")
                use_dve = evict == "dve" or (
                    evict == "split" and (m * nt + n) % 2 == 1
                )
                if use_dve:
                    nc.vector.tensor_scalar_add(
                        ot[:], ps[:], bias_sb[:, m : m + 1]
                    )
                else:
                    nc.scalar.activation(
                        ot[:],
                        ps[:],
                        mybir.ActivationFunctionType.Identity,
                        bias=bias_sb[:, m : m + 1],
                    )
                getattr(nc, out_eng).dma_start(
                    out=out_d[
                        m * 128 : (m + 1) * 128, n * 512 : (n + 1) * 512
                    ],
                    in_=ot[:],
                )

            def _load_w(m):
                w_m = wpool.tile([128, KT * 128], pe_dt, name=f"wm{m}", tag="wm")
                if layout == "pslab":
                    nc.sync.dma_start(
                        out=w_m[:], in_=wt_d[m * 128 : (m + 1) * 128, :]
                    )
                    return w_m
                w_v = w_m[:].rearrange("p (t f) -> p t f", t=KT)
                wkc = KT // w_split
                for i in range(w_split):
                    nc.sync.dma_start(
                        out=w_v[:, i * wkc : (i + 1) * wkc, :],
                        in_=wt_v[
                            :, i * wkc : (i + 1) * wkc,
                            m * 128 : (m + 1) * 128,
                        ],
                    )
                return w_m

            # repeats > 1 re-runs the identical computation inside one NEFF;
            # used for benchmarking (HW time = delta between repeat counts).
            for _rep in range(repeats):
                x_engine = getattr(nc, x_eng)
                x_cs = []
                for i in range(XCH):
                    x_c = xpool.tile(
                        [128, kc * b_sh], pe_dt, name=f"xc{i}", tag=f"xc{i}"
                    )
                    x_cs.append(x_c)
                    if layout == "pslab":
                        x_engine.dma_start(
                            out=x_c[:],
                            in_=xt_d[:, i * kc * b_sh : (i + 1) * kc * b_sh],
                        )
                    else:
                        x_engine.dma_start(
                            out=x_c[:].rearrange("p (t f) -> p t f", t=kc),
                            in_=xt_v[:, i * kc : (i + 1) * kc, :],
                        )

                if loop_order == "kphase":
                    # Phase A: while x streams in, run the first PH m-tiles'
                    # partial-k accumulations chunk by chunk across PH psum
                    # banks, so the PE has PH*kc matmuls per arriving x chunk
                    # instead of kc.  Phase B: remaining m-tiles normally.
                    assert nt == 1
                    PH = min(mt, ps_bufs)
                    w_ts = {m: _load_w(m) for m in range(mt)}
                    _load_bias()
                    ps_l = [
                        pspool.tile([128, 512], f32, name=f"psA{m}", tag="ps")
                        for m in range(PH)
                    ]
                    for c in range(XCH):
                        for m in range(PH):
                            for k in range(c * kc, (c + 1) * kc):
                                _mm(ps_l[m], w_ts[m], m, 0, k)
                    for m in range(PH):
                        _evict(ps_l[m], m, 0)
                    for m in range(PH, mt):
                        ps = pspool.tile([128, 512], f32, name=f"psB{m}", tag="ps")
                        for k in range(KT):
                            _mm(ps, w_ts[m], m, 0, k)
                        _evict(ps, m, 0)
                else:
                    _load_bias()
                    for m in range(mt):
                        w_m = _load_w(m)
                        for n in range(nt):
                            ps = pspool.tile(
                                [128, 512], f32, name=f"psm{m}_{n}", tag="ps"
                            )
                            for k in range(KT):
                                _mm(ps, w_m, m, n, k)
                            _evict(ps, m, n)

    nc.compile()
    _prog_cache[key] = nc
    return nc


def _build_packed(pe_dtype_name, bg, og, repeats, xch, x_eng, ps_bufs,
                  out_eng, out_fp16, wch, loop_order="mnk"):
    """Variant with x and w host-packed into their exact SBUF layouts
    ([128, KT*free]) so every DMA moves contiguous >=2KB per-partition rows,
    and the whole per-core weight stays SBUF-resident (no per-m streaming)."""
    import concourse.tile as tile
    from concourse import bacc, mybir

    b_sh, out_sh, mt, nt = _shapes(bg, og)
    pe_dt = getattr(mybir.dt, pe_dtype_name)
    f32 = mybir.dt.float32
    out_dt = pe_dt if out_fp16 else f32

    nc = bacc.Bacc(
        "TRN2", target_bir_lowering=False, debug=False, num_devices=N_CORES
    )

    xt_d = nc.dram_tensor("xt", [128, KT * b_sh], pe_dt, kind="ExternalInput").ap()
    wt_d = nc.dram_tensor("wt", [128, KT * out_sh], pe_dt, kind="ExternalInput").ap()
    bias_d = nc.dram_tensor("biaspm", [128, mt], f32, kind="ExternalInput").ap()
    out_d = nc.dram_tensor("outT", [out_sh, b_sh], out_dt, kind="ExternalOutput").ap()

    with tile.TileContext(nc) as tc:
        with (
            tc.tile_pool(name="xsb", bufs=2) as xpool,
            tc.tile_pool(name="wsb", bufs=2) as wpool,
            tc.tile_pool(name="bsb", bufs=1) as bpool,
            tc.tile_pool(name="osb", bufs=4) as opool,
            tc.tile_pool(name="ps", bufs=ps_bufs, space="PSUM") as pspool,
        ):
            bias_sb = bpool.tile([128, mt], f32)
            nc.gpsimd.dma_start(out=bias_sb[:], in_=bias_d[:])

            for _rep in range(repeats):
                x_sb = xpool.tile([128, KT * b_sh], pe_dt)
                xc = KT * b_sh // xch
                x_engine = getattr(nc, x_eng)
                for i in range(xch):
                    x_engine.dma_start(
                        out=x_sb[:, i * xc : (i + 1) * xc],
                        in_=xt_d[:, i * xc : (i + 1) * xc],
                    )
                w_sb = wpool.tile([128, KT * out_sh], pe_dt)
                wc = KT * out_sh // wch
                for i in range(wch):
                    nc.sync.dma_start(
                        out=w_sb[:, i * wc : (i + 1) * wc],
                        in_=wt_d[:, i * wc : (i + 1) * wc],
                    )

                def _evict(ps, m, n):
                    ot = opool.tile([128, 512], out_dt)
                    nc.scalar.activation(
                        ot[:],
                        ps[:],
                        mybir.ActivationFunctionType.Identity,
                        bias=bias_sb[:, m : m + 1],
                    )
                    getattr(nc, out_eng).dma_start(
                        out=out_d[
                            m * 128 : (m + 1) * 128, n * 512 : (n + 1) * 512
                        ],
                        in_=ot[:],
                    )

                def _mm(ps, m, n, k):
                    nc.tensor.matmul(
                        ps[:],
                        lhsT=w_sb[
                            :, k * out_sh + m * 128 : k * out_sh + (m + 1) * 128
                        ],
                        rhs=x_sb[
                            :, k * b_sh + n * 512 : k * b_sh + (n + 1) * 512
                        ],
                        start=(k == 0),
                        stop=(k == KT - 1),
                    )

                for m in range(mt):
                    if loop_order == "mkn":
                        # same stationary weight feeds all n back-to-back
                        ps_list = [
                            pspool.tile([128, 512], f32, name=f"psn{n}", tag="ps")
                            for n in range(nt)
                        ]
                        for k in range(KT):
                            for n in range(nt):
                                _mm(ps_list[n], m, n, k)
                        for n in range(nt):
                            _evict(ps_list[n], m, n)
                    else:
                        for n in range(nt):
                            ps = pspool.tile([128, 512], f32)
                            for k in range(KT):
                                _mm(ps, m, n, k)
                            _evict(ps, m, n)

    nc.compile()
    return nc


def _pack_sbuf_layout(a, free):
    """[KT*128, free] -> [128, KT*free] matching the SBUF tile layout."""
    return np.ascontiguousarray(
        a.reshape(KT, 128, free).transpose(1, 0, 2).reshape(128, KT * free)
    )


def _prep_host(x, weight, bias, indices, pe_dtype_name=None, bg=None, og=None,
               layout="pslab"):
    from concourse import mybir

    pe_dtype_name = pe_dtype_name or PE_DTYPE
    bg = bg or BG
    og = og or OG
    b_sh, out_sh, mt, nt = _shapes(bg, og)
    np_dt = mybir.dt.np(getattr(mybir.dt, pe_dtype_name))

    x = np.asarray(x, dtype=np.float32)
    weight = np.asarray(weight, dtype=np.float32)
    bias = np.asarray(bias, dtype=np.float32)
    idx = np.asarray(indices).astype(np.int64)

    counts = np.bincount(idx, minlength=IN_F).astype(np.float32)
    w_t = np.ascontiguousarray((weight * counts[None, :]).T).astype(np_dt)  # [in, out]
    xt_full = np.ascontiguousarray(x.T).astype(np_dt)  # [in, B]

    w_cache = {}

    def _w_for(ogi):
        # identical for every core sharing an out-feature group — pack once
        if ogi not in w_cache:
            wt_c = np.ascontiguousarray(
                w_t[:, ogi * out_sh : (ogi + 1) * out_sh]
            )
            if layout == "packed":
                wt_c = _pack_sbuf_layout(wt_c, out_sh)
            elif layout == "pslab":
                # slab-major: wp[m*128+p, t*128+j] = w_t[t*128+p, m*128+j]
                wt_c = np.ascontiguousarray(
                    wt_c.reshape(KT, 128, mt, 128)
                    .transpose(2, 1, 0, 3)
                    .reshape(mt * 128, KT * 128)
                )
            w_cache[ogi] = wt_c
        return w_cache[ogi]

    in_maps = []
    for c in range(N_CORES):
        bgi, ogi = divmod(c, og)
        bias_blk = bias[ogi * out_sh : (ogi + 1) * out_sh]
        xt_c = np.ascontiguousarray(xt_full[:, bgi * b_sh : (bgi + 1) * b_sh])
        if layout in ("packed", "pslab"):
            xt_c = _pack_sbuf_layout(xt_c, b_sh)
        in_maps.append(
            {
                "xt": xt_c,
                "wt": _w_for(ogi),
                "biaspm": np.ascontiguousarray(bias_blk.reshape(mt, 128).T),
            }
        )
    return in_maps


def _gather_out(results, bg=None, og=None):
    bg = bg or BG
    og = og or OG
    b_sh, out_sh, mt, nt = _shapes(bg, og)
    out = np.empty((B, OUT_F), dtype=np.float32)
    for c in range(N_CORES):
        bgi, ogi = divmod(c, og)
        out[
            bgi * b_sh : (bgi + 1) * b_sh, ogi * out_sh : (ogi + 1) * out_sh
        ] = results[c]["outT"].T.astype(np.float32)
    return out


_runner_cache = {}


def _get_runner(nc):
    """Cached jitted executor for the compiled bass program (the
    run_bass_kernel_spmd axon path rebuilds its jit closure per call;
    caching avoids re-tracing on repeat kernel() invocations)."""
    if id(nc) in _runner_cache:
        return _runner_cache[id(nc)]

    import jax
    from jax.sharding import Mesh, PartitionSpec
    from jax.experimental.shard_map import shard_map
    import concourse.mybir as mybir
    from concourse.bass2jax import (
        _bass_exec_p,
        install_neuronx_cc_hook,
        partition_id_tensor,
    )

    install_neuronx_cc_hook()
    partition_name = nc.partition_id_tensor.name if nc.partition_id_tensor else None
    in_names, out_names, out_avals, zero_shapes = [], [], [], []
    for alloc in nc.m.functions[0].allocations:
        if not isinstance(alloc, mybir.MemoryLocationSet):
            continue
        name = alloc.memorylocations[0].name
        if alloc.kind == "ExternalInput":
            if name != partition_name:
                in_names.append(name)
        elif alloc.kind == "ExternalOutput":
            out_names.append(name)
            shape = tuple(alloc.tensor_shape)
            dtype = mybir.dt.np(alloc.dtype)
            out_avals.append(jax.core.ShapedArray(shape, dtype))
            zero_shapes.append((shape, dtype))
    all_in_names = list(in_names) + list(out_names)
    if partition_name is not None:
        all_in_names.append(partition_name)

    def _body(*args):
        operands = list(args)
        if partition_name is not None:
            operands.append(partition_id_tensor())
        return tuple(
            _bass_exec_p.bind(
                *operands,
                out_avals=tuple(out_avals),
                in_names=tuple(all_in_names),
                out_names=tuple(out_names),
                lowering_input_output_aliases=(),
                sim_require_finite=True,
                sim_require_nnan=True,
                nc=nc,
            )
        )

    devices = jax.devices()[:N_CORES]
    mesh = Mesh(np.asarray(devices), ("core",))
    n_io = len(in_names) + len(out_names)
    fn = jax.jit(
        shard_map(
            _body,
            mesh=mesh,
            in_specs=(PartitionSpec("core"),) * n_io,
            out_specs=(PartitionSpec("core"),) * len(out_names),
            check_rep=False,
        ),
        keep_unused=True,
    )

    runner = (fn, in_names, out_names, out_avals, zero_shapes)
    _runner_cache[id(nc)] = runner
    return runner


def _run_fast(nc, in_maps):
    fn, in_names, out_names, out_avals, zero_shapes = _get_runner(nc)
    concat_in = [
        np.concatenate([np.asarray(m[name]) for m in in_maps], axis=0)
        for name in in_names
    ]
    concat_zeros = [
        np.zeros((N_CORES * s[0], *s[1:]), dt) for s, dt in zero_shapes
    ]
    outs = fn(*concat_in, *concat_zeros)
    return [
        {
            name: np.asarray(outs[i]).reshape(N_CORES, *out_avals[i].shape)[c]
            for i, name in enumerate(out_names)
        }
        for c in range(N_CORES)
    ]


def kernel(x, weight, bias, indices):
    global LAST_RESULTS

    in_maps = _prep_host(x, weight, bias, indices)
    nc = _build_program()
    try:
        results = _run_fast(nc, in_maps)
    except Exception:
        from concourse.bass_utils import run_bass_kernel_spmd

        results = run_bass_kernel_spmd(
            nc, in_maps, list(range(N_CORES))
        ).results
    LAST_RESULTS = results
    return _gather_out(results)
